# revision 5
# baseline (speedup 1.0000x reference)
"""Trainium2 Bass kernel for EfficientAttention (linear attention block).

Computation (per batch b, head h):
    qkv = x @ w_qkv.T + b_qkv
    q = softmax(q, axis=head_dim) * head_dim**-0.5
    k = softmax(k, axis=seqlen)
    kv[d,e] = sum_s k[s,d] v[s,e]          (per-head 64x64 state)
    out[s,e] = sum_d q[s,d] kv[d,e]
    y = out @ w_proj.T + b_proj

Sharding: 8 cores = (batch b = c//2, seq half = c%2); 2048 tokens per core,
all 16 heads. Cross-core coupling: kv state + k-softmax denominator Z ->
one AllReduce (pairs of cores) of [128, 520] fp32.

v3 design (v1 452us -> v2 342us -> v3):
- All matmuls bf16 (tolerance 2e-2 >> bf16 GEMM error ~4e-3). Halves DMA
  and SBUF so all four weight matrices stay resident and x is loaded once.
- kv state accumulated TRANSPOSED (kvT[e,d] = v^T @ ek, one [128,128]
  matmul per head-pair) so the proj fold M[hd,:] = sum_e KV[h,d,e] WpT[he,:]
  needs no on-chip transpose; y = qT @ M replaces BOTH the attention matmul
  and out@WpT. 1/Z row-scaling rides M's PSUM->SBUF copy (ACT per-partition
  scalar); head_dim**-0.5 folded into wp on the host.
- Phase 1 software-pipelined: v-projection lags k by 2 token-blocks, the
  kv-state matmuls by 3, so PE never waits on ACT's exp/copy and the wv
  weight load gets extra time at startup.
- q^T via the XBAR DMA transpose (one InstDmaTransposeAnt per token block,
  [128,1024]bf16 -> qtall [d, pair, t]) - frees the PE transposes and the
  DVE PSUM copies.
- Startup: only wk/wv/x chunk0 load eagerly (3 queues); wq/wp issue from
  the scalar stream a few token-blocks into phase 1 to not steal HBM
  bandwidth from the critical path.
"""

import os
import sys

sys.path.insert(0, "/opt/trn_rl_repo")

import numpy as np

import concourse.bacc as bacc
import concourse.tile as tile
from concourse import mybir
from concourse import bass_utils

F32 = mybir.dt.float32
BF16 = mybir.dt.bfloat16

D = 1024          # model dim (= qkv contraction dim)
T = 2048          # tokens per core (one batch element's half sequence)
NH = 16           # heads
HD = 64           # head dim
NPAIR = 8         # head pairs (2 heads / 128 partitions)
KC = D // 128     # contraction chunks of 128
TB = T // 128     # token blocks of 128
SCALE = HD ** -0.5

N_CORES = 8

VLAG = 2          # v-projection lags k by this many token blocks
KVLAG = 3         # kv-state matmuls lag k by this many token blocks


def build_program(with_bias=True):
    nc = bacc.Bacc("TRN2", target_bir_lowering=False, num_devices=N_CORES)

    xt = nc.dram_tensor("xt", [D, T], BF16, kind="ExternalInput")      # x chunk, transposed
    wq = nc.dram_tensor("wq", [D, D], BF16, kind="ExternalInput")      # w_q.T
    wk = nc.dram_tensor("wk", [D, D], BF16, kind="ExternalInput")      # w_k.T
    wv = nc.dram_tensor("wv", [D, D], BF16, kind="ExternalInput")      # w_v.T
    wp = nc.dram_tensor("wp", [D, D], BF16, kind="ExternalInput")      # w_proj.T * SCALE
    bq = nc.dram_tensor("bq", [D], F32, kind="ExternalInput")
    bk = nc.dram_tensor("bk", [D], F32, kind="ExternalInput")
    bv = nc.dram_tensor("bv", [D], F32, kind="ExternalInput")
    bp = nc.dram_tensor("bp", [D], F32, kind="ExternalInput")
    cst = nc.dram_tensor("cst", [128, 132], BF16, kind="ExternalInput")  # identity | ones | pad
    y = nc.dram_tensor("y", [T, D], F32, kind="ExternalOutput")

    xt_v = xt.rearrange("(kc p) t -> p kc t", p=128)
    wq_v = wq.rearrange("(kc p) f -> p kc f", p=128)
    wk_v = wk.rearrange("(kc p) f -> p kc f", p=128)
    wv_v = wv.rearrange("(kc p) f -> p kc f", p=128)
    wp_v = wp.rearrange("(kc p) f -> p kc f", p=128)

    def bias_bcast(b):
        import concourse.bass as bass
        ap = b[:]
        return bass.AP(tensor=ap.tensor, offset=ap.offset, ap=[[0, 128]] + list(ap.ap))

    with tile.TileContext(nc) as tc:
        with (
            tc.tile_pool(name="const", bufs=1) as const,
            tc.tile_pool(name="wpool", bufs=1) as wpool,
            tc.tile_pool(name="xin", bufs=1) as xin,
            tc.tile_pool(name="ekv", bufs=4) as ekv,
            tc.tile_pool(name="acc", bufs=1) as accp,
            tc.tile_pool(name="qpool", bufs=2) as qpool,
            tc.tile_pool(name="qt", bufs=1) as qtpool,
            tc.tile_pool(name="kvsb", bufs=1) as kvsbp,
            tc.tile_pool(name="yout", bufs=4) as youtp,
            tc.tile_pool(name="psum", bufs=5, space="PSUM") as psum,
            tc.tile_pool(name="dram", bufs=1, space="DRAM") as dram,
        ):
            # ---- SBUF allocations ----
            cst_sb = const.tile([128, 132], BF16, tag="cst")
            wk_sb = wpool.tile([128, KC, D], BF16, tag="wk")
            wv_sb = wpool.tile([128, KC, D], BF16, tag="wv")
            wq_sb = wpool.tile([128, KC, D], BF16, tag="wq")
            wp_sb = wpool.tile([128, KC, D], BF16, tag="wp")
            xt_sb = xin.tile([128, KC, T], BF16, tag="xt")

            # ---- startup DMAs: critical-first across the three DMA-capable
            # queues (sync/scalar/gpsimd). wq/wp are deferred into phase 1.
            nc.sync.dma_start(cst_sb, cst[:])
            nc.sync.dma_start(wk_sb[:, 0:4, :], wk_v[:, 0:4, :])
            nc.scalar.dma_start(xt_sb[:, :, 0:512], xt_v[:, :, 0:512])
            nc.scalar.dma_start(wk_sb[:, 4:8, :], wk_v[:, 4:8, :])
            nc.gpsimd.dma_start(wv_sb[:, 0:4, :], wv_v[:, 0:4, :])
            nc.gpsimd.dma_start(wv_sb[:, 4:8, :], wv_v[:, 4:8, :])
            nc.sync.dma_start(xt_sb[:, :, 512:1024], xt_v[:, :, 512:1024])

            ident = cst_sb[:, 0:128]
            ones = cst_sb[:, 128:129]

            if with_bias:
                bk_sb = const.tile([128, D], BF16, tag="bk")
                bv_sb = const.tile([128, D], BF16, tag="bv")
                bq_sb = const.tile([128, D], BF16, tag="bq")
                bp_sb = const.tile([128, D], F32, tag="bp")
                nc.gpsimd.dma_start(bk_sb, bias_bcast(bk))
                nc.gpsimd.dma_start(bv_sb, bias_bcast(bv))
                nc.gpsimd.dma_start(bq_sb, bias_bcast(bq))
                nc.gpsimd.dma_start(bp_sb, bias_bcast(bp))

            # kvT accumulator in SBUF f32: pair p at cols [128p:128p+128],
            # layout [e', d'] per pair (rows e' = 2 heads x 64; cols d').
            kvacc = accp.tile([128, 1024], F32, tag="kvacc")
            # zero-padded kvT lhsT blocks (built post-collective); memset now
            kvT_sb = kvsbp.tile([128, NPAIR, 128], BF16, tag="kvT")
            nc.vector.memset(kvT_sb[:], 0.0)

            # Z accumulators: ones^T @ ek, PSUM accumulation across tbs
            # (single accumulation group alone in its bank = safe).
            zps = [psum.tile([128, 512], F32, tag=f"z{h}", name=f"zps{h}", bufs=1)
                   for h in range(2)]

            # ---- Phase 1, software-pipelined ----
            eks = [None] * TB
            vvs = [None] * TB

            def k_block(tb):
                xtile = xt_sb[:, :, tb * 128:(tb + 1) * 128]
                ek = ekv.tile([128, D], BF16, tag="ek", name=f"ek{tb}")
                eks[tb] = ek
                for half in range(2):
                    sl = slice(half * 512, (half + 1) * 512)
                    ps = psum.tile([128, 512], F32, tag="mm")
                    for kc in range(KC):
                        nc.tensor.matmul(ps, xtile[:, kc, :], wk_sb[:, kc, sl],
                                         start=(kc == 0), stop=(kc == KC - 1))
                    if with_bias:
                        nc.vector.tensor_add(ps, ps, bk_sb[:, sl])
                    nc.scalar.activation(ek[:, sl], ps, mybir.ActivationFunctionType.Exp)
                    nc.tensor.matmul(zps[half][0:1, :], ones, ek[:, sl],
                                     start=(tb == 0), stop=(tb == TB - 1))

            def v_block(tb):
                xtile = xt_sb[:, :, tb * 128:(tb + 1) * 128]
                vv = ekv.tile([128, D], BF16, tag="v", name=f"v{tb}")
                vvs[tb] = vv
                for half in range(2):
                    sl = slice(half * 512, (half + 1) * 512)
                    ps = psum.tile([128, 512], F32, tag="mm")
                    for kc in range(KC):
                        nc.tensor.matmul(ps, xtile[:, kc, :], wv_sb[:, kc, sl],
                                         start=(kc == 0), stop=(kc == KC - 1))
                    if with_bias:
                        nc.vector.tensor_add(ps, ps, bv_sb[:, sl])
                    nc.scalar.copy(vv[:, sl], ps)

            def kv_block(tb):
                # pair p -> out [e' 128, d' 128] = vv_p^T @ ek_p
                ek, vv = eks[tb], vvs[tb]
                for g in range(2):
                    kps = psum.tile([128, 512], F32, tag="mm")
                    for j in range(4):
                        p = 4 * g + j
                        nc.tensor.matmul(
                            kps[:, j * 128:(j + 1) * 128],
                            vv[:, p * 128:(p + 1) * 128],
                            ek[:, p * 128:(p + 1) * 128],
                            start=True, stop=True)
                    if tb == 0:
                        nc.vector.tensor_copy(kvacc[:, g * 512:(g + 1) * 512], kps)
                    else:
                        nc.vector.tensor_add(kvacc[:, g * 512:(g + 1) * 512],
                                             kvacc[:, g * 512:(g + 1) * 512], kps)
                eks[tb] = vvs[tb] = None

            for s in range(TB + KVLAG):
                if s < TB:
                    k_block(s)
                if s == 1:
                    nc.sync.dma_start(xt_sb[:, :, 1024:1536], xt_v[:, :, 1024:1536])
                if s == 3:
                    nc.sync.dma_start(xt_sb[:, :, 1536:2048], xt_v[:, :, 1536:2048])
                if s == 2:
                    nc.scalar.dma_start(wq_sb[:, 0:4, :], wq_v[:, 0:4, :])
                if s == 4:
                    nc.scalar.dma_start(wq_sb[:, 4:8, :], wq_v[:, 4:8, :])
                if s == 6:
                    nc.scalar.dma_start(wp_sb[:, 0:4, :], wp_v[:, 0:4, :])
                if s == 8:
                    nc.scalar.dma_start(wp_sb[:, 4:8, :], wp_v[:, 4:8, :])
                if VLAG <= s < TB + VLAG:
                    v_block(s - VLAG)
                if KVLAG <= s:
                    kv_block(s - KVLAG)

            # ---- Z readout ----
            zrow = accp.tile([1, D], F32, tag="zrow")
            for half in range(2):
                sl = slice(half * 512, (half + 1) * 512)
                nc.scalar.copy(zrow[:, sl], zps[half][0:1, :])
            zdram = dram.tile([D], F32, tag="zd")
            nc.sync.dma_start(zdram[:].unsqueeze(0), zrow)

            # ---- stage compacted partial (kvT | Z), AllReduce across seq pair
            # pair p -> cols [64p : 64p+64]; head 2p rows 0:64, 2p+1 rows 64:128
            stage = accp.tile([128, 520], F32, tag="stage")
            for p in range(NPAIR):
                nc.vector.tensor_copy(stage[0:64, 64 * p:64 * p + 64],
                                      kvacc[0:64, 128 * p:128 * p + 64])
                nc.vector.tensor_copy(stage[64:128, 64 * p:64 * p + 64],
                                      kvacc[64:128, 128 * p + 64:128 * p + 128])
            nc.sync.dma_start(stage[:, 512:520],
                              zdram[:].rearrange("(g p) -> p g", p=128))
            cin = dram.tile([128, 520], F32, tag="cin")
            cout = dram.tile([128, 520], F32, tag="cout")
            nc.sync.dma_start(cin, stage)
            nc.gpsimd.collective_compute(
                "AllReduce", mybir.AluOpType.add,
                replica_groups=[[0, 1], [2, 3], [4, 5], [6, 7]],
                ins=[cin[:].opt()], outs=[cout[:].opt()])
            kvred = accp.tile([128, 520], F32, tag="kvred")
            nc.sync.dma_start(kvred, cout)

            # ---- q sweep (overlaps the collective; no dependency on it) ----
            qtall = qtpool.tile([128, NPAIR, T], BF16, tag="qtall")
            for tb in range(TB):
                eq = qpool.tile([128, D], BF16, tag="eq")
                eqn = qpool.tile([128, D], BF16, tag="eqn")
                for half in range(2):
                    sl = slice(half * 512, (half + 1) * 512)
                    ps = psum.tile([128, 512], F32, tag="mm")
                    for kc in range(KC):
                        nc.tensor.matmul(ps, xt_sb[:, kc, tb * 128:(tb + 1) * 128],
                                         wq_sb[:, kc, sl],
                                         start=(kc == 0), stop=(kc == KC - 1))
                    if with_bias:
                        nc.vector.tensor_add(ps, ps, bq_sb[:, sl])
                    nc.scalar.activation(eq[:, sl], ps, mybir.ActivationFunctionType.Exp)
                sums = qpool.tile([128, NH], F32, tag="sums")
                nc.vector.reduce_sum(sums, eq[:].rearrange("p (h e) -> p h e", e=HD),
                                     axis=mybir.AxisListType.X)
                rfac = qpool.tile([128, NH], F32, tag="rfac")
                nc.vector.reciprocal(rfac, sums)
                for h in range(NH):
                    nc.vector.tensor_scalar_mul(eqn[:, h * HD:(h + 1) * HD],
                                                eq[:, h * HD:(h + 1) * HD],
                                                rfac[:, h:h + 1])
                # XBAR DMA transpose: qtall[d, pair, t] = eqn[t, 128*pair + d]
                nc.scalar.dma_start_transpose(
                    qtall[:, :, tb * 128:(tb + 1) * 128], eqn[:])

            # ---- build kvT lhsT blocks + M = kvT^T @ wp (row-scaled by 1/Z)
            rz = accp.tile([128, NPAIR], F32, tag="rz")
            nc.vector.reciprocal(rz, kvred[:, 512:520])
            for p in range(NPAIR):
                nc.vector.tensor_copy(kvT_sb[0:64, p, 0:64],
                                      kvred[0:64, 64 * p:64 * p + 64])
                nc.vector.tensor_copy(kvT_sb[64:128, p, 64:128],
                                      kvred[64:128, 64 * p:64 * p + 64])
            m_sb = kvsbp.tile([128, NPAIR, D], BF16, tag="m")
            for p in range(NPAIR):
                for oc in range(2):
                    sl = slice(oc * 512, (oc + 1) * 512)
                    mps = psum.tile([128, 512], F32, tag="mm")
                    nc.tensor.matmul(mps, kvT_sb[:, p, :], wp_sb[:, p, sl],
                                     start=True, stop=True)
                    nc.scalar.mul(m_sb[:, p, sl], mps, rz[:, p:p + 1])

            # ---- y sweep: y = qT^T @ M (+ bp), token-major stores ----
            for tb in range(TB):
                for oc in range(2):
                    sl = slice(oc * 512, (oc + 1) * 512)
                    ps = psum.tile([128, 512], F32, tag="mm")
                    for kc in range(KC):
                        nc.tensor.matmul(ps, qtall[:, kc, tb * 128:(tb + 1) * 128],
                                         m_sb[:, kc, sl],
                                         start=(kc == 0), stop=(kc == KC - 1))
                    yt = youtp.tile([128, 512], F32, tag="y")
                    if with_bias:
                        nc.vector.tensor_add(yt, ps, bp_sb[:, sl])
                    else:
                        nc.scalar.copy(yt, ps)
                    nc.sync.dma_start(y[tb * 128:(tb + 1) * 128, sl], yt)

    nc.compile()
    return nc


_NC = {}


def _get_nc(with_bias=False):
    if with_bias not in _NC:
        _NC[with_bias] = build_program(with_bias=with_bias)
    return _NC[with_bias]


def kernel(x, w_qkv, b_qkv, w_proj, b_proj):
    import ml_dtypes
    bf16 = ml_dtypes.bfloat16

    x = np.asarray(x, dtype=np.float32)
    w_qkv = np.asarray(w_qkv, dtype=np.float32)
    b_qkv = np.asarray(b_qkv, dtype=np.float32)
    w_proj = np.asarray(w_proj, dtype=np.float32)
    b_proj = np.asarray(b_proj, dtype=np.float32)

    bs, seqlen, dim = x.shape
    half = seqlen // 2

    wq = np.ascontiguousarray(w_qkv[0:D].T.astype(bf16))
    wk = np.ascontiguousarray(w_qkv[D:2 * D].T.astype(bf16))
    wv = np.ascontiguousarray(w_qkv[2 * D:3 * D].T.astype(bf16))
    wp = np.ascontiguousarray((w_proj.T * SCALE).astype(bf16))
    bq, bk, bv = b_qkv[0:D], b_qkv[D:2 * D], b_qkv[2 * D:3 * D]

    cst = np.concatenate(
        [np.eye(128, dtype=np.float32),
         np.ones((128, 1), dtype=np.float32),
         np.zeros((128, 3), dtype=np.float32)], axis=1).astype(bf16)

    in_maps = []
    for c in range(N_CORES):
        b, s = divmod(c, 2)
        chunk = np.ascontiguousarray(x[b, s * half:(s + 1) * half, :].T.astype(bf16))
        in_maps.append({
            "xt": chunk, "wq": wq, "wk": wk, "wv": wv, "wp": wp,
            "bq": np.ascontiguousarray(bq), "bk": np.ascontiguousarray(bk),
            "bv": np.ascontiguousarray(bv), "bp": np.ascontiguousarray(b_proj),
            "cst": cst,
        })

    with_bias = bool(np.any(b_qkv)) or bool(np.any(b_proj))
    nc = _get_nc(with_bias)
    global _last_in_maps, _last_with_bias
    _last_in_maps = in_maps
    _last_with_bias = with_bias
    res = bass_utils.run_bass_kernel_spmd(nc, in_maps, core_ids=list(range(N_CORES)))

    out = np.empty((bs, seqlen, dim), dtype=np.float32)
    for c in range(N_CORES):
        b, s = divmod(c, 2)
        out[b, s * half:(s + 1) * half, :] = res.results[c]["y"]
    return out


# revision 10
# speedup vs baseline: 1.0362x; 1.0362x over previous
"""Trainium2 Bass kernel for EfficientAttention (linear attention block).

Computation (per batch b, head h):
    qkv = x @ w_qkv.T + b_qkv
    q = softmax(q, axis=head_dim) * head_dim**-0.5
    k = softmax(k, axis=seqlen)
    kv[d,e] = sum_s k[s,d] v[s,e]          (per-head 64x64 state)
    out[s,e] = sum_d q[s,d] kv[d,e]
    y = out @ w_proj.T + b_proj

Sharding: 8 cores = (batch b = c//2, seq half = c%2); 2048 tokens per core,
all 16 heads. Cross-core coupling: kv state + k-softmax denominator Z ->
one AllReduce (pairs of cores) of [128, 520] fp32.

v3 design (v1 452us -> v2 342us -> v3):
- All matmuls bf16 (tolerance 2e-2 >> bf16 GEMM error ~4e-3). Halves DMA
  and SBUF so all four weight matrices stay resident and x is loaded once.
- kv state accumulated TRANSPOSED (kvT[e,d] = v^T @ ek, one [128,128]
  matmul per head-pair) so the proj fold M[hd,:] = sum_e KV[h,d,e] WpT[he,:]
  needs no on-chip transpose; y = qT @ M replaces BOTH the attention matmul
  and out@WpT. 1/Z row-scaling rides M's PSUM->SBUF copy (ACT per-partition
  scalar); head_dim**-0.5 folded into wp on the host.
- Phase 1 software-pipelined: v-projection lags k by 2 token-blocks, the
  kv-state matmuls by 3, so PE never waits on ACT's exp/copy and the wv
  weight load gets extra time at startup.
- q^T via PE transposes (the XBAR DMA transpose serializes behind the
  triggered collective on the dynamic-DMA path and runs ~4us/block - 3x
  slower than the PE; measured, not modeled).
- Startup: only wk/wv/x chunk0 load eagerly (3 queues); wq/wp issue from
  the scalar stream a few token-blocks into phase 1 to not steal HBM
  bandwidth from the critical path.
"""

import os
import sys

sys.path.insert(0, "/opt/trn_rl_repo")

import numpy as np

import concourse.bacc as bacc
import concourse.tile as tile
from concourse import mybir
from concourse import bass_utils

F32 = mybir.dt.float32
BF16 = mybir.dt.bfloat16

D = 1024          # model dim (= qkv contraction dim)
T = 2048          # tokens per core (one batch element's half sequence)
NH = 16           # heads
HD = 64           # head dim
NPAIR = 8         # head pairs (2 heads / 128 partitions)
KC = D // 128     # contraction chunks of 128
TB = T // 128     # token blocks of 128
SCALE = HD ** -0.5

N_CORES = 8

VLAG = 2          # v-projection lags k by this many token blocks
KVLAG = 3         # kv-state matmuls lag k by this many token blocks


def build_program(with_bias=True):
    nc = bacc.Bacc("TRN2", target_bir_lowering=False, num_devices=N_CORES)

    xt = nc.dram_tensor("xt", [D, T], BF16, kind="ExternalInput")      # x chunk, transposed
    wq = nc.dram_tensor("wq", [D, D], BF16, kind="ExternalInput")      # w_q.T
    wk = nc.dram_tensor("wk", [D, D], BF16, kind="ExternalInput")      # w_k.T
    wv = nc.dram_tensor("wv", [D, D], BF16, kind="ExternalInput")      # w_v.T
    wp = nc.dram_tensor("wp", [D, D], BF16, kind="ExternalInput")      # w_proj.T * SCALE
    bq = nc.dram_tensor("bq", [D], F32, kind="ExternalInput")
    bk = nc.dram_tensor("bk", [D], F32, kind="ExternalInput")
    bv = nc.dram_tensor("bv", [D], F32, kind="ExternalInput")
    bp = nc.dram_tensor("bp", [D], F32, kind="ExternalInput")
    cst = nc.dram_tensor("cst", [128, 132], BF16, kind="ExternalInput")  # identity | ones | pad
    y = nc.dram_tensor("y", [T, D], F32, kind="ExternalOutput")

    xt_v = xt.rearrange("(kc p) t -> p kc t", p=128)
    wq_v = wq.rearrange("(kc p) f -> p kc f", p=128)
    wk_v = wk.rearrange("(kc p) f -> p kc f", p=128)
    wv_v = wv.rearrange("(kc p) f -> p kc f", p=128)
    wp_v = wp.rearrange("(kc p) f -> p kc f", p=128)

    def bias_bcast(b):
        import concourse.bass as bass
        ap = b[:]
        return bass.AP(tensor=ap.tensor, offset=ap.offset, ap=[[0, 128]] + list(ap.ap))

    with tile.TileContext(nc) as tc:
        with (
            tc.tile_pool(name="const", bufs=1) as const,
            tc.tile_pool(name="wpool", bufs=1) as wpool,
            tc.tile_pool(name="xin", bufs=1) as xin,
            tc.tile_pool(name="ekv", bufs=4) as ekv,
            tc.tile_pool(name="acc", bufs=1) as accp,
            tc.tile_pool(name="qpool", bufs=3) as qpool,
            tc.tile_pool(name="qt", bufs=1) as qtpool,
            tc.tile_pool(name="kvsb", bufs=1) as kvsbp,
            tc.tile_pool(name="yout", bufs=4) as youtp,
            tc.tile_pool(name="psum", bufs=4, space="PSUM") as psum,
            tc.tile_pool(name="dram", bufs=1, space="DRAM") as dram,
        ):
            # ---- SBUF allocations ----
            cst_sb = const.tile([128, 132], BF16, tag="cst")
            wk_sb = wpool.tile([128, KC, D], BF16, tag="wk")
            wv_sb = wpool.tile([128, KC, D], BF16, tag="wv")
            wq_sb = wpool.tile([128, KC, D], BF16, tag="wq")
            wp_sb = wpool.tile([128, KC, D], BF16, tag="wp")
            xt_sb = xin.tile([128, KC, T], BF16, tag="xt")

            # ---- startup DMAs: critical-first across the three DMA-capable
            # queues (sync/scalar/gpsimd). wq/wp are deferred into phase 1.
            nc.sync.dma_start(cst_sb, cst[:])
            nc.sync.dma_start(wk_sb[:, 0:4, :], wk_v[:, 0:4, :])
            nc.scalar.dma_start(wk_sb[:, 4:8, :], wk_v[:, 4:8, :])
            nc.gpsimd.dma_start(xt_sb[:, :, 0:512], xt_v[:, :, 0:512])
            nc.gpsimd.dma_start(wv_sb[:, 0:4, :], wv_v[:, 0:4, :])
            nc.gpsimd.dma_start(wv_sb[:, 4:8, :], wv_v[:, 4:8, :])
            nc.sync.dma_start(xt_sb[:, :, 512:1024], xt_v[:, :, 512:1024])

            ident = cst_sb[:, 0:128]
            ones = cst_sb[:, 128:129]

            if with_bias:
                bk_sb = const.tile([128, D], BF16, tag="bk")
                bv_sb = const.tile([128, D], BF16, tag="bv")
                bq_sb = const.tile([128, D], BF16, tag="bq")
                bp_sb = const.tile([128, D], F32, tag="bp")
                nc.gpsimd.dma_start(bk_sb, bias_bcast(bk))
                nc.gpsimd.dma_start(bv_sb, bias_bcast(bv))
                nc.gpsimd.dma_start(bq_sb, bias_bcast(bq))
                nc.gpsimd.dma_start(bp_sb, bias_bcast(bp))

            # kvT accumulator in SBUF f32: pair p at cols [128p:128p+128],
            # layout [e', d'] per pair (rows e' = 2 heads x 64; cols d').
            kvacc = accp.tile([128, 1024], F32, tag="kvacc")
            # zero-padded kvT lhsT blocks (built post-collective); memset now
            kvT_sb = kvsbp.tile([128, NPAIR, 128], BF16, tag="kvT")
            nc.vector.memset(kvT_sb[:], 0.0)

            # Z accumulators: ones^T @ ek, PSUM accumulation across tbs
            # (single accumulation group alone in its bank = safe).
            zps = [psum.tile([128, 512], F32, tag=f"z{h}", name=f"zps{h}", bufs=1)
                   for h in range(2)]

            # ---- Phase 1, software-pipelined ----
            eks = [None] * TB
            vvs = [None] * TB

            def k_block(tb):
                xtile = xt_sb[:, :, tb * 128:(tb + 1) * 128]
                ek = ekv.tile([128, D], BF16, tag="ek", name=f"ek{tb}")
                eks[tb] = ek
                for half in range(2):
                    sl = slice(half * 512, (half + 1) * 512)
                    ps = psum.tile([128, 512], F32, tag="mm")
                    for kc in range(KC):
                        nc.tensor.matmul(ps, xtile[:, kc, :], wk_sb[:, kc, sl],
                                         start=(kc == 0), stop=(kc == KC - 1))
                    if with_bias:
                        nc.vector.tensor_add(ps, ps, bk_sb[:, sl])
                    nc.scalar.activation(ek[:, sl], ps, mybir.ActivationFunctionType.Exp)
                    nc.tensor.matmul(zps[half][0:1, :], ones, ek[:, sl],
                                     start=(tb == 0), stop=(tb == TB - 1))

            def v_block(tb):
                xtile = xt_sb[:, :, tb * 128:(tb + 1) * 128]
                vv = ekv.tile([128, D], BF16, tag="v", name=f"v{tb}")
                vvs[tb] = vv
                for half in range(2):
                    sl = slice(half * 512, (half + 1) * 512)
                    ps = psum.tile([128, 512], F32, tag="mm")
                    for kc in range(KC):
                        nc.tensor.matmul(ps, xtile[:, kc, :], wv_sb[:, kc, sl],
                                         start=(kc == 0), stop=(kc == KC - 1))
                    if with_bias:
                        nc.vector.tensor_add(ps, ps, bv_sb[:, sl])
                    nc.scalar.copy(vv[:, sl], ps)

            def kv_block(tb):
                # pair p -> out [e' 128, d' 128] = vv_p^T @ ek_p
                ek, vv = eks[tb], vvs[tb]
                for g in range(2):
                    kps = psum.tile([128, 512], F32, tag="mm")
                    for j in range(4):
                        p = 4 * g + j
                        nc.tensor.matmul(
                            kps[:, j * 128:(j + 1) * 128],
                            vv[:, p * 128:(p + 1) * 128],
                            ek[:, p * 128:(p + 1) * 128],
                            start=True, stop=True)
                    if tb == 0:
                        nc.vector.tensor_copy(kvacc[:, g * 512:(g + 1) * 512], kps)
                    else:
                        nc.vector.tensor_add(kvacc[:, g * 512:(g + 1) * 512],
                                             kvacc[:, g * 512:(g + 1) * 512], kps)
                eks[tb] = vvs[tb] = None

            for s in range(TB + KVLAG):
                if s < TB:
                    k_block(s)
                if s == 1:
                    nc.sync.dma_start(xt_sb[:, :, 1024:1536], xt_v[:, :, 1024:1536])
                if s == 3:
                    nc.sync.dma_start(xt_sb[:, :, 1536:2048], xt_v[:, :, 1536:2048])
                if s == 2:
                    nc.scalar.dma_start(wq_sb[:, 0:4, :], wq_v[:, 0:4, :])
                if s == 4:
                    nc.scalar.dma_start(wq_sb[:, 4:8, :], wq_v[:, 4:8, :])
                if s == 6:
                    nc.scalar.dma_start(wp_sb[:, 0:4, :], wp_v[:, 0:4, :])
                if s == 8:
                    nc.scalar.dma_start(wp_sb[:, 4:8, :], wp_v[:, 4:8, :])
                if VLAG <= s < TB + VLAG:
                    v_block(s - VLAG)
                if KVLAG <= s:
                    kv_block(s - KVLAG)

            # ---- Z readout ----
            zrow = accp.tile([1, D], F32, tag="zrow")
            for half in range(2):
                sl = slice(half * 512, (half + 1) * 512)
                nc.scalar.copy(zrow[:, sl], zps[half][0:1, :])
            zdram = dram.tile([D], F32, tag="zd")
            nc.sync.dma_start(zdram[:].unsqueeze(0), zrow)

            # ---- stage compacted partial (kvT | Z), AllReduce across seq pair
            # pair p -> cols [64p : 64p+64]; head 2p rows 0:64, 2p+1 rows 64:128
            stage = accp.tile([128, 520], F32, tag="stage")
            for p in range(NPAIR):
                nc.vector.tensor_copy(stage[0:64, 64 * p:64 * p + 64],
                                      kvacc[0:64, 128 * p:128 * p + 64])
                nc.vector.tensor_copy(stage[64:128, 64 * p:64 * p + 64],
                                      kvacc[64:128, 128 * p + 64:128 * p + 128])
            nc.sync.dma_start(stage[:, 512:520],
                              zdram[:].rearrange("(g p) -> p g", p=128))
            cin = dram.tile([128, 520], F32, tag="cin")
            cout = dram.tile([128, 520], F32, tag="cout")
            nc.sync.dma_start(cin, stage)
            nc.gpsimd.collective_compute(
                "AllReduce", mybir.AluOpType.add,
                replica_groups=[[0, 1], [2, 3], [4, 5], [6, 7]],
                ins=[cin[:].opt()], outs=[cout[:].opt()])
            kvred = accp.tile([128, 520], F32, tag="kvred")
            nc.sync.dma_start(kvred, cout)

            # ---- q sweep (overlaps the collective; no dependency on it) ----
            qtall = qtpool.tile([128, NPAIR, T], BF16, tag="qtall")
            for tb in range(TB):
                eq = qpool.tile([128, D], BF16, tag="eq")
                eqn = qpool.tile([128, D], BF16, tag="eqn")
                for half in range(2):
                    sl = slice(half * 512, (half + 1) * 512)
                    ps = psum.tile([128, 512], F32, tag="mm")
                    for kc in range(KC):
                        nc.tensor.matmul(ps, xt_sb[:, kc, tb * 128:(tb + 1) * 128],
                                         wq_sb[:, kc, sl],
                                         start=(kc == 0), stop=(kc == KC - 1))
                    if with_bias:
                        nc.vector.tensor_add(ps, ps, bq_sb[:, sl])
                    nc.scalar.activation(eq[:, sl], ps, mybir.ActivationFunctionType.Exp)
                sums = qpool.tile([128, NH], F32, tag="sums")
                nc.vector.reduce_sum(sums, eq[:].rearrange("p (h e) -> p h e", e=HD),
                                     axis=mybir.AxisListType.X)
                rfac = qpool.tile([128, NH], F32, tag="rfac")
                nc.vector.reciprocal(rfac, sums)
                for h in range(NH):
                    nc.vector.tensor_scalar_mul(eqn[:, h * HD:(h + 1) * HD],
                                                eq[:, h * HD:(h + 1) * HD],
                                                rfac[:, h:h + 1])
                for g4 in range(2):
                    tp = psum.tile([128, 512], BF16, tag="tr", bufs=2)
                    for j in range(4):
                        p = 4 * g4 + j
                        nc.tensor.transpose(tp[:, j * 128:(j + 1) * 128],
                                            eqn[:, p * 128:(p + 1) * 128], ident)
                    nc.vector.tensor_copy(
                        qtall[:, 4 * g4:4 * g4 + 4, tb * 128:(tb + 1) * 128],
                        tp[:].rearrange("p (j t) -> p j t", j=4))

            # ---- build kvT lhsT blocks + M = kvT^T @ wp (row-scaled by 1/Z)
            rz = accp.tile([128, NPAIR], F32, tag="rz")
            nc.vector.reciprocal(rz, kvred[:, 512:520])
            for p in range(NPAIR):
                nc.vector.tensor_copy(kvT_sb[0:64, p, 0:64],
                                      kvred[0:64, 64 * p:64 * p + 64])
                nc.vector.tensor_copy(kvT_sb[64:128, p, 64:128],
                                      kvred[64:128, 64 * p:64 * p + 64])
            m_sb = kvsbp.tile([128, NPAIR, D], BF16, tag="m")
            for p in range(NPAIR):
                for oc in range(2):
                    sl = slice(oc * 512, (oc + 1) * 512)
                    mps = psum.tile([128, 512], F32, tag="mm")
                    nc.tensor.matmul(mps, kvT_sb[:, p, :], wp_sb[:, p, sl],
                                     start=True, stop=True)
                    nc.scalar.mul(m_sb[:, p, sl], mps, rz[:, p:p + 1])

            # ---- y sweep: y = qT^T @ M (+ bp), token-major stores ----
            for tb in range(TB):
                for oc in range(2):
                    sl = slice(oc * 512, (oc + 1) * 512)
                    ps = psum.tile([128, 512], F32, tag="mm")
                    for kc in range(KC):
                        nc.tensor.matmul(ps, qtall[:, kc, tb * 128:(tb + 1) * 128],
                                         m_sb[:, kc, sl],
                                         start=(kc == 0), stop=(kc == KC - 1))
                    yt = youtp.tile([128, 512], F32, tag="y")
                    if with_bias:
                        nc.vector.tensor_add(yt, ps, bp_sb[:, sl])
                    else:
                        nc.scalar.copy(yt, ps)
                    nc.sync.dma_start(y[tb * 128:(tb + 1) * 128, sl], yt)

    nc.compile()
    return nc


_NC = {}


def _get_nc(with_bias=False):
    if with_bias not in _NC:
        _NC[with_bias] = build_program(with_bias=with_bias)
    return _NC[with_bias]


def kernel(x, w_qkv, b_qkv, w_proj, b_proj):
    import ml_dtypes
    bf16 = ml_dtypes.bfloat16

    x = np.asarray(x, dtype=np.float32)
    w_qkv = np.asarray(w_qkv, dtype=np.float32)
    b_qkv = np.asarray(b_qkv, dtype=np.float32)
    w_proj = np.asarray(w_proj, dtype=np.float32)
    b_proj = np.asarray(b_proj, dtype=np.float32)

    bs, seqlen, dim = x.shape
    half = seqlen // 2

    wq = np.ascontiguousarray(w_qkv[0:D].T.astype(bf16))
    wk = np.ascontiguousarray(w_qkv[D:2 * D].T.astype(bf16))
    wv = np.ascontiguousarray(w_qkv[2 * D:3 * D].T.astype(bf16))
    wp = np.ascontiguousarray((w_proj.T * SCALE).astype(bf16))
    bq, bk, bv = b_qkv[0:D], b_qkv[D:2 * D], b_qkv[2 * D:3 * D]

    cst = np.concatenate(
        [np.eye(128, dtype=np.float32),
         np.ones((128, 1), dtype=np.float32),
         np.zeros((128, 3), dtype=np.float32)], axis=1).astype(bf16)

    in_maps = []
    for c in range(N_CORES):
        b, s = divmod(c, 2)
        chunk = np.ascontiguousarray(x[b, s * half:(s + 1) * half, :].T.astype(bf16))
        in_maps.append({
            "xt": chunk, "wq": wq, "wk": wk, "wv": wv, "wp": wp,
            "bq": np.ascontiguousarray(bq), "bk": np.ascontiguousarray(bk),
            "bv": np.ascontiguousarray(bv), "bp": np.ascontiguousarray(b_proj),
            "cst": cst,
        })

    with_bias = bool(np.any(b_qkv)) or bool(np.any(b_proj))
    nc = _get_nc(with_bias)
    global _last_in_maps, _last_with_bias
    _last_in_maps = in_maps
    _last_with_bias = with_bias
    res = bass_utils.run_bass_kernel_spmd(nc, in_maps, core_ids=list(range(N_CORES)))

    out = np.empty((bs, seqlen, dim), dtype=np.float32)
    for c in range(N_CORES):
        b, s = divmod(c, 2)
        out[b, s * half:(s + 1) * half, :] = res.results[c]["y"]
    return out
